# revision 32
# baseline (speedup 1.0000x reference)
"""Trainium2 Bass kernel for nn_BMManager_76476187673212.

Computation (matches the reference nn.Module):
  1. dropout(x, p=0.1) with a fixed jax PRNG key (42) -> keep mask is
     input-independent, so it is folded into x on the host (x*keep, and
     the 1/(1-p) scale folded into W)
  2. h = einsum('bsd,gd->bsg', x_dropped, W) + b
  3. global (detached) stats: noise = mean(h)/10 * 0.5 + std(h,ddof=1)/5 * z
  4. h += noise
  5. segment forward-fill along s driven by critic_mask

Sharding: pure data parallel, batch dim (32) split over 8 cores (4 rows
each, T=16384 tokens per core).

Everything on device stays in [G, token] layout (G=128 on partitions);
the output is written as [G, T] fp16 and transposed to [tok, G] on the
host, so the device does zero transposes.

Per-core device pipeline (per 1024-token chunk):
  DMA xT fp16 + m (=not-start) u8
  -> PE: 8 fp16 matmuls (K=128 each) -> PSUM h (f32)
  -> ACT: copy PSUM->SBUF fp16 with per-partition bias b and S1 accum
  -> ACT: square pass with S2 accum
  -> GPSIMD: d1 = (m==0)*h  (segment-start premultiply, one STT op)
  -> DVE: tensor_tensor_scan  state = m*state + d1 (exact forward fill,
     chained across chunks via the previous chunk's last column)
stats: free-dim reduce -> AllReduce([128,2]) -> partition_all_reduce
  -> every partition holds global (S1,S2); noise computed columnar as
     noise_col[g] = sqrt(var)*z[g]/5 + 0.05*mean -> [128,1]
tail (per chunk): out = ff + noise_col (per-partition scalar add,
  alternating DVE/gpsimd/ACT) -> DMA to [G, T] fp16
"""

import os
import sys

sys.path.insert(0, "/opt/trn_rl_repo")

import numpy as np

import concourse.bacc as bacc
import concourse.mybir as mybir
import concourse.tile as tile
from concourse.bass_utils import run_bass_kernel_spmd

F32 = mybir.dt.float32
F16 = mybir.dt.float16
FP8 = mybir.dt.float8e4
U8 = mybir.dt.uint8

N_CORES = 8
B, S, D, G = 32, 4096, 512, 128
T = (B // N_CORES) * S          # tokens per core = 16384
C = 2048                         # tokens per chunk
NCHUNK = T // C                  # 8
KCH = D // 128                   # 4 contraction chunks
MM = 512                         # matmul moving-operand width (ISA max)
N_TOTAL = float(B * S * G)       # stats element count
DOUT_P = 0.1
MEAN_FACTOR = 10.0
STD_FACTOR = 5.0

_compiled = {}


def _build_program(with_collective=True):
    nc = bacc.Bacc("TRN2", target_bir_lowering=False, debug=False,
                   num_devices=N_CORES)

    xt_in = nc.dram_tensor("xt", [D, T], F16, kind="ExternalInput").ap()
    # plane 0: m (=not start) scan multiplier; plane 1: s (=start)
    m_in = nc.dram_tensor("mrep", [128, 2, T], U8, kind="ExternalInput").ap()
    wt_in = nc.dram_tensor("wt", [D, G], F16, kind="ExternalInput").ap()
    b_in = nc.dram_tensor("bcol", [G, 1], F32, kind="ExternalInput").ap()
    z_in = nc.dram_tensor("zcol", [G, 1], F32, kind="ExternalInput").ap()
    out_d = nc.dram_tensor("out", [G, T], F16, kind="ExternalOutput").ap()

    xt_v = xt_in.rearrange("(k p) t -> p k t", k=KCH, p=128)

    with tile.TileContext(nc) as tc:
        with (
            tc.tile_pool(name="per", bufs=1) as per,
            tc.tile_pool(name="ldx", bufs=4) as ldx,
            tc.tile_pool(name="ldm", bufs=3) as ldm,
            tc.tile_pool(name="hh", bufs=3) as hh,
            tc.tile_pool(name="dd", bufs=2) as dd,
            tc.tile_pool(name="sq", bufs=2) as sqp,
            tc.tile_pool(name="os", bufs=3) as osp,
            tc.tile_pool(name="ps", bufs=2, space="PSUM") as ps,
            tc.tile_pool(name="dram", bufs=1, space="DRAM") as dram,
        ):
            # ---------- persistent setup ----------
            ff = per.tile([128, T], F16)           # forward-filled h
            sum_buf = per.tile([128, NCHUNK], F32)
            sumsq_buf = per.tile([128, NCHUNK], F32)

            wt_f = per.tile([128, KCH, G], F16)
            nc.sync.dma_start(
                wt_f[:], wt_in.rearrange("(k p) g -> p k g", k=KCH, p=128))
            b_col = per.tile([128, 1], F32)
            nc.sync.dma_start(b_col[:], b_in[:])
            z_col = per.tile([128, 1], F32)        # already z/STD_FACTOR
            nc.sync.dma_start(z_col[:], z_in[:])

            warm = per.tile([1, 2], F32)
            nc.gpsimd.memset(warm[:], 1.0)
            nc.scalar.sqrt(warm[:], warm[:])
            ones_col = per.tile([128, 1], F32)
            nc.gpsimd.memset(ones_col[:], 1.0)

            # warm-up collective: pays the CC ring setup cost up front and
            # acts as a cross-core barrier, so the real stats AllReduce at
            # the end of phase A is cheap
            if with_collective:
                wu_in = dram.tile([1, 2], F32)
                wu_out = dram.tile([1, 2], F32)
                nc.sync.dma_start(wu_in[:], warm[:])
                nc.gpsimd.collective_compute(
                    "AllReduce", mybir.AluOpType.add,
                    replica_groups=[list(range(N_CORES))],
                    ins=[wu_in[:].opt()], outs=[wu_out[:].opt()])

            # ---------- phase A ----------
            for c in range(NCHUNK):
                ts = slice(c * C, (c + 1) * C)
                xt_t = ldx.tile([128, KCH, C], F16, name="xt_t")
                m2_t = ldm.tile([128, 2, C], U8, name="m2_t")
                # per-k DMAs: the first matmul can start after 1/4 of the
                # chunk's data has landed
                for k in range(KCH):
                    nc.sync.dma_start(xt_t[:, k, :], xt_v[:, k, ts])
                nc.sync.dma_start(m2_t[:], m_in[:, :, ts])
                m_t = m2_t[:, 0, :]
                s_t = m2_t[:, 1, :]

                # k outer: stationary weights reused across the C/MM halves
                hps = ps.tile([128, C], F32, name="hps")
                for k in range(KCH):
                    for half in range(C // MM):
                        hs = slice(half * MM, (half + 1) * MM)
                        nc.tensor.matmul(
                            hps[:, hs], wt_f[:, k, :], xt_t[:, k, hs],
                            start=(k == 0), stop=(k == KCH - 1))

                # h -> SBUF fp16 with bias; S1/S2 via ACT accumulators
                h_sb = hh.tile([128, C], F16, name="h_sb")
                nc.scalar.activation(
                    h_sb[:], hps[:], mybir.ActivationFunctionType.Identity,
                    bias=b_col[:], accum_out=sum_buf[:, c:c + 1])
                sq_sb = sqp.tile([128, C], FP8, name="sq_sb")
                nc.scalar.activation(
                    sq_sb[:], h_sb[:], mybir.ActivationFunctionType.Square,
                    accum_out=sumsq_buf[:, c:c + 1])

                # stats merge + collective, emitted BEFORE the last chunk's
                # d1/scan so the CC trigger isn't stuck behind them in the
                # gpsimd queue. ACT accum merges the free dim, a ones-matmul
                # the partition dim (ACT and PE drain early).
                if c == NCHUNK - 1:
                    s12 = per.tile([128, 2], F32)
                    sdump = per.tile([128, NCHUNK], F32)
                    nc.scalar.activation(sdump[:], sum_buf[:],
                                         mybir.ActivationFunctionType.Copy,
                                         accum_out=s12[:, 0:1])
                    nc.scalar.activation(sdump[:], sumsq_buf[:],
                                         mybir.ActivationFunctionType.Copy,
                                         accum_out=s12[:, 1:2])
                    sps = ps.tile([2, 1], F32, name="sps", tag="hps")
                    nc.tensor.matmul(sps[:], s12[:], ones_col[:],
                                     start=True, stop=True)
                    ssb = per.tile([2, 1], F32)
                    nc.scalar.copy(ssb[:], sps[:])
                    cc_in = dram.tile([2, 1], F32)
                    cc_out = dram.tile([2, 1], F32)
                    nc.sync.dma_start(cc_in[:], ssb[:])
                    if with_collective:
                        nc.gpsimd.collective_compute(
                            "AllReduce", mybir.AluOpType.add,
                            replica_groups=[list(range(N_CORES))],
                            ins=[cc_in[:].opt()], outs=[cc_out[:].opt()])
                    else:
                        nc.sync.dma_start(cc_out[:], cc_in[:])

                # forward fill: d1 = s*h ; state = m*state + d1
                d1_t = dd.tile([128, C], F16, name="d1_t")
                nc.gpsimd.tensor_mul(d1_t[:], s_t, h_sb[:])
                init = 0.0 if c == 0 else ff[:, c * C - 1:c * C]
                nc.vector.tensor_tensor_scan(
                    ff[:, ts], m_t, d1_t[:], init,
                    mybir.AluOpType.mult, mybir.AluOpType.add)

            # ---------- noise from global stats ----------
            sgp = per.tile([1, 2], F32)
            nc.sync.dma_start(sgp[:], cc_out[:].rearrange("a b -> b a"))
            s12g = per.tile([128, 2], F32)   # every partition: global S1,S2
            nc.gpsimd.partition_broadcast(s12g[:], sgp[:])

            # noise_col = sqrt((S2-S1^2/N)/(N-1)) * (z/5) + S1*0.05/N
            t1 = per.tile([128, 1], F32)     # S1^2/N
            nc.vector.scalar_tensor_tensor(
                t1[:], s12g[:, 0:1], 1.0 / N_TOTAL, s12g[:, 0:1],
                mybir.AluOpType.mult, mybir.AluOpType.mult)
            u = per.tile([128, 1], F32)      # S2 - S1^2/N
            nc.vector.tensor_sub(u[:], s12g[:, 1:2], t1[:])
            sig = per.tile([128, 1], F32)
            nc.scalar.activation(sig[:], u[:],
                                 mybir.ActivationFunctionType.Sqrt,
                                 scale=1.0 / (N_TOTAL - 1.0))
            c1 = per.tile([128, 1], F32)
            nc.vector.tensor_scalar_mul(
                c1[:], s12g[:, 0:1], 0.5 / (MEAN_FACTOR * N_TOTAL))
            noise_col = per.tile([128, 1], F32)
            nc.scalar.activation(noise_col[:], z_col[:],
                                 mybir.ActivationFunctionType.Identity,
                                 bias=c1[:], scale=sig[:])
            # noise broadcast tile: nb = 0*ff + noise_col
            nb = per.tile([128, C], F16)
            nc.scalar.activation(nb[:], ff[:, 0:C],
                                 mybir.ActivationFunctionType.Identity,
                                 bias=noise_col[:], scale=0.0)

            # ---------- tail: add noise + store ----------
            for c in range(NCHUNK):
                ts = slice(c * C, (c + 1) * C)
                o_sb = osp.tile([128, C], F16, name="o_sb")
                nc.vector.tensor_add(o_sb[:], ff[:, ts], nb[:])
                nc.sync.dma_start(out_d[:, ts], o_sb[:])

    nc.compile()
    return nc


_RNG_CODE = """
import os, site
for _p in os.environ.get("NIX_PYTHONPATH", "").split(os.pathsep):
    if _p:
        site.addsitedir(_p)
import numpy as np, jax, jax.numpy as jnp
kd, kn = jax.random.split(jax.random.key(42))
keep = jax.random.bernoulli(kd, 1.0 - {p}, ({b}, {s}, {d}))
z = jax.random.normal(kn, ({g},), dtype=jnp.float32)
np.save({out!r} + "/keep.npy", np.asarray(keep))
np.save({out!r} + "/z.npy", np.asarray(z))
"""


def _fixed_rng():
    """Dropout mask + noise vector from the model's fixed PRNG key (42).

    Computed with jax itself (bit-exact vs the reference) in a true-CPU
    subprocess: `-S` skips the axon sitecustomize and PYTHONPATH is
    stripped, otherwise jax in this environment binds to the
    axon/neuron backend whose threefry bits differ from CPU.
    """
    import shutil
    import subprocess
    import tempfile

    tmp = tempfile.mkdtemp()
    code = _RNG_CODE.format(p=DOUT_P, b=B, s=S, d=D, g=G, out=tmp)
    env = {k: v for k, v in os.environ.items() if k != "PYTHONPATH"}
    env["JAX_PLATFORMS"] = "cpu"
    py = shutil.which("python3") or sys.executable
    subprocess.run([py, "-S", "-c", code], env=env, check=True,
                   capture_output=True)
    keep = np.load(tmp + "/keep.npy")
    z = np.load(tmp + "/z.npy")
    return keep, z


def _host_prep(x, critic_mask, W, b):
    keep, z = _fixed_rng()

    # x*keep, transposed to [D, T] fp16 per core (1/(1-p) lives in W)
    xk = x.reshape(N_CORES, T, D)
    kk = keep.reshape(N_CORES, T, D)
    xt = np.empty((N_CORES, D, T), dtype=np.float16)
    for c in range(N_CORES):
        xt[c] = np.where(kk[c], xk[c], 0.0).T.astype(np.float16)

    starts = np.ones((B, S), dtype=bool)
    starts[:, 1:] = critic_mask[:, :-1]
    ms = np.stack([~starts, starts], axis=1).astype(np.uint8)  # [B, 2, S]
    rows = B // N_CORES
    ms = (ms.reshape(N_CORES, rows, 2, S).transpose(0, 2, 1, 3)
          .reshape(N_CORES, 1, 2, T))
    mrep = np.ascontiguousarray(np.broadcast_to(ms, (N_CORES, 128, 2, T)))

    wt = np.ascontiguousarray(
        (W.T.astype(np.float32) / (1.0 - DOUT_P)).astype(np.float16))
    bcol = np.asarray(b, dtype=np.float32).reshape(G, 1)
    zcol = (np.asarray(z, dtype=np.float32) / STD_FACTOR).reshape(G, 1)
    return xt, mrep, wt, bcol, zcol


def _run(x, critic_mask, W, b, **spmd_kwargs):
    x = np.asarray(x, dtype=np.float32)
    critic_mask = np.asarray(critic_mask, dtype=bool)
    W = np.asarray(W, dtype=np.float32)
    b = np.asarray(b, dtype=np.float32)

    xt, mrep, wt, bcol, zcol = _host_prep(x, critic_mask, W, b)

    if "nc" not in _compiled:
        _compiled["nc"] = _build_program()
    nc = _compiled["nc"]

    in_maps = [
        {"xt": xt[c], "mrep": mrep[c], "wt": wt, "bcol": bcol, "zcol": zcol}
        for c in range(N_CORES)
    ]
    res = run_bass_kernel_spmd(nc, in_maps, list(range(N_CORES)), **spmd_kwargs)
    # device output is [G, T] fp16 per core -> [B, S, G] f32
    out = np.empty((B, S, G), dtype=np.float32)
    rows = B // N_CORES
    for c in range(N_CORES):
        o = np.asarray(res.results[c]["out"])       # [G, T] fp16
        out[c * rows:(c + 1) * rows] = (
            o.reshape(G, rows, S).transpose(1, 2, 0).astype(np.float32))
    return out, res


def kernel(x, critic_mask, W, b):
    out, _ = _run(x, critic_mask, W, b)
    return out
